# revision 9
# baseline (speedup 1.0000x reference)
"""Trainium2 Bass kernel for nn_DA_conv (dynamic depthwise conv + CA attention).

Data-parallel over batch: 16 samples / 8 cores = 2 samples per core.
Partition layout: 128 partitions = (sample s in 0..1) x (channel c in 0..63).

Per-core pipeline over the 128x128 image (free dim = h*128+w, 16384 cols):
  - feat fp32 DMA'd in 2048-col blocks; DVE converts into a W-padded fp16
    image buffer pad1 [128, 128*130] (zero cols at w=-1, w=128 per row).
  - dynamic 3x3 depthwise conv: per-partition tap scalars kern_p[128, 9]
    computed on-device from deg via small PE GEMMs.
    Blocks 0..NPE-1: all 9 taps as PE diagonal-matmul accumulation into PSUM
      (lhsT = diag(kern_p[:, tap]) fp16, rhs = shifted pad1 view).
    Remaining blocks: taps on DVE/ACT (init tensor_scalar 4x, some
      scalar_tensor_tensor 1x, some ACT-mul + DVE tensor_tensor-add 2x).
  - leaky-relu(0.1) via ACT Prelu -> act16.
  - 1x1 conv: PE matmul with block-diag(W_conv.T) fp16, + residual
    att*feat via PE diag(att) matmul into the same PSUM accumulation.
  - epilogue: ACT Identity(psum + b_conv) fp32 -> SBUF -> DMA out.

kernel(**inputs) takes FULL numpy inputs, returns FULL [16,64,128,128] f32.
"""
import numpy as np
from contextlib import ExitStack

import concourse.bass as bass
import concourse.tile as tile
from concourse import bacc, mybir
from concourse.bass_utils import run_bass_kernel_spmd

F16 = mybir.dt.float16
F32 = mybir.dt.float32
AF = mybir.ActivationFunctionType
OP = mybir.AluOpType

N_CORES = 8
B, C, H, W = 16, 64, 128, 128
BC = B // N_CORES          # 2 samples per core
P = BC * C                 # 128 partitions
HW = H * W                 # 16384
DEG, RED = 512, 8
K = 3
WP = W + 2                 # padded row stride (130)
BLK = 2048                 # block cols (16 image rows)
NBLK = HW // BLK           # 8
ROWS_PER_BLK = BLK // W    # 16
NPE = 5                    # blocks 0..NPE-1 use PE taps; rest DVE/ACT
# tap order: first tap must be di=0 (full coverage); init DVE tap is (0,-1)
TAPS = [(0, -1), (0, 0), (0, 1), (-1, -1), (-1, 0), (-1, 1), (1, -1), (1, 0), (1, 1)]
# non-PE blocks: which taps go ACT-assisted (mul on ACT, add on DVE 2x).
AA_TAPS = {(0, 0), (-1, 0), (1, 0)}   # the 3 odd-offset taps (stt is 1x anyway;
                                      # keep DVE for even ones)

_CACHE = {}


def _tap_idx(di, dj):
    return TAPS.index((di, dj))


def _build():
    nc = bacc.Bacc("TRN2", target_bir_lowering=False, debug=False,
                   num_devices=N_CORES)
    feat = nc.declare_dram_parameter("feat", [BC, C, H, W], F32, isOutput=False)
    deg = nc.declare_dram_parameter("deg", [BC, DEG, 64], F32, isOutput=False)
    wcat = nc.declare_dram_parameter("wcat", [DEG, 128], F32, isOutput=False)
    wk1t = nc.declare_dram_parameter("wk1t", [C, RED], F32, isOutput=False)
    wk2t = nc.declare_dram_parameter("wk2t", [RED, C * K * K], F32, isOutput=False)
    wdu1t = nc.declare_dram_parameter("wdu1t", [C, RED], F32, isOutput=False)
    wdu2t = nc.declare_dram_parameter("wdu2t", [RED, C], F32, isOutput=False)
    w2blk = nc.declare_dram_parameter("w2blk", [P, P], F16, isOutput=False)
    bias_p = nc.declare_dram_parameter("bias_p", [P, 1], F32, isOutput=False)
    eye16 = nc.declare_dram_parameter("eye16", [P, P], F16, isOutput=False)
    out = nc.declare_dram_parameter("out", [BC, C, H, W], F32, isOutput=True)

    featv = feat.ap().rearrange("s c h w -> (s c) (h w)")
    outv = out.ap().rearrange("s c h w -> (s c) (h w)")
    kern2_dram = nc.dram_tensor("kern2_tmp", [BC, C * K * K], F32)
    att_dram = nc.dram_tensor("att_tmp", [C, BC], F32)

    with tile.TileContext(nc) as tc:
        with ExitStack() as ctx:
            # ---------------- persistent pools ----------------
            const = ctx.enter_context(tc.tile_pool(name="const", bufs=1))
            padp = ctx.enter_context(tc.tile_pool(name="padp", bufs=1))

            pad1 = padp.tile([P, H * WP], F16)       # W-padded fp16 image
            pad1v = pad1[:].rearrange("p (h w) -> p h w", w=WP)

            w2blk_sb = const.tile([P, P], F16)
            nc.sync.dma_start(w2blk_sb[:], w2blk.ap())
            bias_sb = const.tile([P, 1], F32)
            nc.sync.dma_start(bias_sb[:], bias_p.ap())
            eye_sb = const.tile([P, P], F16)
            nc.sync.dma_start(eye_sb[:], eye16.ap())
            wcat_sb = const.tile([128, 4 * 128], F32)
            nc.sync.dma_start(
                wcat_sb[:].rearrange("p (t m) -> p t m", t=4),
                wcat.ap().rearrange("(t p) m -> p t m", p=128))
            wk1t_sb = const.tile([C, RED], F32)
            nc.sync.dma_start(wk1t_sb[:], wk1t.ap())
            wk2t_sb = const.tile([RED, C * K * K], F32)
            nc.sync.dma_start(wk2t_sb[:], wk2t.ap())
            wdu1t_sb = const.tile([C, RED], F32)
            nc.sync.dma_start(wdu1t_sb[:], wdu1t.ap())
            wdu2t_sb = const.tile([RED, C], F32)
            nc.sync.dma_start(wdu2t_sb[:], wdu2t.ap())

            kern_p = const.tile([P, K * K], F32)      # per-partition tap scalars
            att_p = const.tile([P, 1], F32)
            diag16 = const.tile([P, K * K * P], F16)  # 9 diag matrices
            attd16 = const.tile([P, P], F16)

            # zero the pad columns (w=0 and w=129 of each padded row)
            nc.vector.memset(pad1v[:, :, 0:1], 0.0)
            nc.vector.memset(pad1v[:, :, WP - 1:WP], 0.0)

            # PE warmup: dense back-to-back matmuls so the HAM clock gate
            # flips to 8/8 (2.4 GHz) ~3.4us in, instead of ~40us into the
            # kernel when the first dense tap work arrives.
            with ExitStack() as wctx:
                wps = wctx.enter_context(
                    tc.tile_pool(name="wps", bufs=1, space="PSUM"))
                wsb = wctx.enter_context(tc.tile_pool(name="wsb", bufs=1))
                wp = wps.tile([P, 512], F32)
                wl = wsb.tile([P, P], F16)
                wr = wsb.tile([P, 512], F16)
                nc.vector.memset(wl[:], 0.0)
                nc.vector.memset(wr[:], 0.0)
                for i in range(24):
                    nc.tensor.matmul(wp[:], wl[:], wr[:], start=True, stop=True)

            # ---------------- prologue: small GEMM chain ----------------
            with ExitStack() as pctx:
                pro = pctx.enter_context(tc.tile_pool(name="pro", bufs=1))
                pps = pctx.enter_context(
                    tc.tile_pool(name="pps", bufs=1, space="PSUM"))

                dg = pro.tile([128, 2 * 256], F32)
                for s in range(BC):
                    nc.sync.dma_start(
                        dg[:, s * 256:(s + 1) * 256].rearrange(
                            "p (t f) -> p t f", t=4),
                        deg.ap()[s].rearrange("(t p) f -> p t f", p=128))
                # dvec[s, d=t*128+p] = mean_f deg -> dv[p, s*4+t]
                dv = pro.tile([128, 8], F32)
                nc.vector.tensor_reduce(
                    dv[:], dg[:].rearrange("p (s t f) -> p s t f", s=2, f=64),
                    axis=mybir.AxisListType.X, op=OP.add)
                nc.vector.tensor_scalar_mul(dv[:], dv[:], 1.0 / 64.0)
                dvv = dv[:].rearrange("p (s t) -> p t s", t=4)

                # f/fa = dvec @ [W_size|W_ac].T : psum [128, 2]
                pf = pps.tile([128, 2], F32)
                for t in range(4):
                    nc.tensor.matmul(pf[:], wcat_sb[:, t * 128:(t + 1) * 128],
                                     dvv[:, t, :], start=(t == 0), stop=(t == 3))
                f_sb = pro.tile([C, 2], F32)
                nc.scalar.activation(f_sb[:], pf[0:C, :], AF.Copy)
                fa_sb = pro.tile([C, 2], F32)
                nc.scalar.activation(fa_sb[:], pf[C:2 * C, :], AF.Copy)

                # kern chain
                ph1 = pps.tile([RED, 2], F32)
                nc.tensor.matmul(ph1[:], wk1t_sb[:], f_sb[:], start=True, stop=True)
                h1l = pro.tile([RED, 2], F32)
                nc.scalar.activation(h1l[:], ph1[:], AF.Prelu, alpha=0.1)
                pk1 = pps.tile([2, 512], F32)
                nc.tensor.matmul(pk1[:], h1l[:], wk2t_sb[:, 0:512],
                                 start=True, stop=True)
                pk2 = pps.tile([2, 64], F32)
                nc.tensor.matmul(pk2[:], h1l[:], wk2t_sb[:, 512:576],
                                 start=True, stop=True)
                kern2 = pro.tile([2, 576], F32)
                nc.scalar.activation(kern2[:, 0:512], pk1[:], AF.Copy)
                nc.scalar.activation(kern2[:, 512:576], pk2[:], AF.Copy)
                nc.sync.dma_start(kern2_dram.ap(), kern2[:])
                for s in range(BC):
                    nc.sync.dma_start(
                        kern_p[s * C:(s + 1) * C, :],
                        kern2_dram.ap()[s].rearrange("(c t) -> c t", t=9))

                # attention chain
                ph2 = pps.tile([RED, 2], F32)
                nc.tensor.matmul(ph2[:], wdu1t_sb[:], fa_sb[:], start=True, stop=True)
                h2l = pro.tile([RED, 2], F32)
                nc.scalar.activation(h2l[:], ph2[:], AF.Prelu, alpha=0.1)
                pat = pps.tile([C, 2], F32)
                nc.tensor.matmul(pat[:], wdu2t_sb[:], h2l[:], start=True, stop=True)
                att_sb = pro.tile([C, 2], F32)
                nc.scalar.activation(att_sb[:], pat[:], AF.Sigmoid)
                nc.sync.dma_start(att_dram.ap(), att_sb[:])
                for s in range(BC):
                    nc.sync.dma_start(att_p[s * C:(s + 1) * C, :],
                                      att_dram.ap()[:, s:s + 1])

                # build diag matrices: diag16[:, t*128+j] = eye * kern_p[:, t]
                for t in range(9):
                    nc.vector.tensor_scalar(
                        diag16[:, t * P:(t + 1) * P], eye_sb[:],
                        kern_p[:, t:t + 1], None, op0=OP.mult)
                nc.vector.tensor_scalar(
                    attd16[:], eye_sb[:], att_p[:], None, op0=OP.mult)

            # ---------------- main loop pools ----------------
            finp = ctx.enter_context(tc.tile_pool(name="finp", bufs=3))
            accp = ctx.enter_context(tc.tile_pool(name="accp", bufs=2))
            tmpp = ctx.enter_context(tc.tile_pool(name="tmpp", bufs=3))
            actp = ctx.enter_context(tc.tile_pool(name="actp", bufs=3))
            outp = ctx.enter_context(tc.tile_pool(name="outp", bufs=3))
            pdwp = ctx.enter_context(tc.tile_pool(name="pdw", bufs=2, space="PSUM"))
            pcvp = ctx.enter_context(tc.tile_pool(name="pcv", bufs=2, space="PSUM"))

            def pad_view(r0, r1, dj):
                """pad1 view of image rows [r0, r1), cols shifted by dj."""
                return pad1v[:, r0:r1, 1 + dj:1 + dj + W]

            # DMA-in + convert for all blocks (scheduler interleaves)
            fins = []
            for b in range(NBLK):
                fin = finp.tile([P, BLK], F32)
                nc.sync.dma_start(fin[:], featv[:, b * BLK:(b + 1) * BLK])
                fins.append(fin)

            for b in range(NBLK):
                r0 = b * ROWS_PER_BLK
                nc.vector.tensor_copy(
                    pad1v[:, r0:r0 + ROWS_PER_BLK, 1:1 + W],
                    fins[b][:].rearrange("p (r w) -> p r w", w=W))

            for b in range(NBLK):
                r0 = b * ROWS_PER_BLK
                r1 = r0 + ROWS_PER_BLK
                is_pe = b < NPE

                if is_pe:
                    # --- PE taps: per 512-col chunk (4 rows) ---
                    act_tiles = []
                    for half in range(2):
                        pdw = pdwp.tile([P, 1024], F32)
                        pdwv = pdw[:].rearrange("p (r w) -> p r w", w=W)
                        for q in range(2):
                            c0 = r0 + half * 8 + q * 4   # first image row of chunk
                            po = q * 4                    # row offset in pdw view
                            for ti, (di, dj) in enumerate(TAPS):
                                a0, a1 = c0 + di, c0 + 4 + di
                                s0, s1 = max(a0, 0), min(a1, H)
                                if s0 >= s1:
                                    continue
                                o0 = po + (s0 - a0)
                                nc.tensor.matmul(
                                    pdwv[:, o0:o0 + (s1 - s0), :],
                                    diag16[:, ti * P:(ti + 1) * P],
                                    pad_view(s0, s1, dj),
                                    start=(ti == 0), stop=(ti == 8))
                        act16 = actp.tile([P, 1024], F16, tag="act")
                        nc.scalar.activation(act16[:], pdw[:], AF.Prelu, alpha=0.1)
                        act_tiles.append(act16)
                else:
                    # --- DVE/ACT taps over the whole 2048 block ---
                    acc = accp.tile([P, BLK], F16)
                    accv = acc[:].rearrange("p (r w) -> p r w", w=W)
                    # init: tap (0,-1), full coverage, tensor_scalar 4x
                    ti0 = _tap_idx(0, -1)
                    nc.vector.tensor_scalar(
                        accv[:], pad_view(r0, r1, -1),
                        kern_p[:, ti0:ti0 + 1], None, op0=OP.mult)
                    for ti, (di, dj) in enumerate(TAPS):
                        if (di, dj) == (0, -1):
                            continue
                        a0, a1 = r0 + di, r1 + di
                        s0, s1 = max(a0, 0), min(a1, H)
                        o0 = s0 - a0 + 0 if a0 >= 0 else s0 - a0
                        o0 = s0 - a0
                        dst = accv[:, o0:o0 + (s1 - s0), :]
                        src = pad_view(s0, s1, dj)
                        if (di, dj) in AA_TAPS:
                            tmp = tmpp.tile([P, BLK], F16, tag="tmp")
                            tmpv = tmp[:].rearrange("p (r w) -> p r w", w=W)
                            tv = tmpv[:, 0:(s1 - s0), :]
                            nc.scalar.activation(
                                tv, src, AF.Copy, scale=kern_p[:, ti:ti + 1])
                            nc.vector.tensor_tensor(dst, dst, tv, op=OP.add)
                        else:
                            nc.vector.scalar_tensor_tensor(
                                dst, src, kern_p[:, ti:ti + 1], dst,
                                op0=OP.mult, op1=OP.add)
                    act16b = actp.tile([P, BLK], F16, tag="actb")
                    nc.scalar.activation(act16b[:], acc[:], AF.Prelu, alpha=0.1)
                    act_tiles = [act16b[:, 0:1024], act16b[:, 1024:2048]]

                # --- 1x1 conv + residual + epilogue (per 1024 half) ---
                ostage = outp.tile([P, BLK], F32)
                for half in range(2):
                    at = act_tiles[half]
                    pcv = pcvp.tile([P, 1024], F32)
                    for q in range(2):
                        c0 = r0 + half * 8 + q * 4
                        nc.tensor.matmul(
                            pcv[:, q * 512:(q + 1) * 512], w2blk_sb[:],
                            at[:, q * 512:(q + 1) * 512] if not is_pe
                            else at[:][:, q * 512:(q + 1) * 512],
                            start=True, stop=False)
                        nc.tensor.matmul(
                            pcv[:, q * 512:(q + 1) * 512], attd16[:],
                            pad_view(c0, c0 + 4, 0),
                            start=False, stop=True)
                    nc.scalar.activation(
                        ostage[:, half * 1024:(half + 1) * 1024], pcv[:],
                        AF.Identity, bias=bias_sb[:], scale=1.0)
                nc.sync.dma_start(outv[:, b * BLK:(b + 1) * BLK], ostage[:])

    nc.compile()
    return nc


def _prep_host(inputs):
    W_size = inputs["W_size"]
    W_ac = inputs["W_ac"]
    W_k1 = inputs["W_k1"]
    W_k2 = inputs["W_k2"]
    W_conv = inputs["W_conv"]
    b_conv = inputs["b_conv"]
    W_du1 = inputs["W_du1"]
    W_du2 = inputs["W_du2"]

    wcat = np.ascontiguousarray(
        np.concatenate([W_size, W_ac], axis=0).T.astype(np.float32))  # [512,128]
    wk1t = np.ascontiguousarray(W_k1.T.astype(np.float32))            # [64,8]
    wk2t = np.ascontiguousarray(W_k2.T.astype(np.float32))            # [8,576]
    wdu1t = np.ascontiguousarray(W_du1.T.astype(np.float32))          # [64,8]
    wdu2t = np.ascontiguousarray(W_du2.T.astype(np.float32))          # [8,64]
    w2blk = np.zeros((P, P), np.float16)
    wct = W_conv.T.astype(np.float16)                                  # [c, o]
    w2blk[0:C, 0:C] = wct
    w2blk[C:2 * C, C:2 * C] = wct
    bias_p = np.tile(b_conv.astype(np.float32), BC).reshape(P, 1)
    eye16 = np.eye(P, dtype=np.float16)
    return dict(wcat=wcat, wk1t=wk1t, wk2t=wk2t, wdu1t=wdu1t, wdu2t=wdu2t,
                w2blk=w2blk, bias_p=np.ascontiguousarray(bias_p), eye16=eye16)


def kernel(**inputs):
    if "nc" not in _CACHE:
        _CACHE["nc"] = _build()
    nc = _CACHE["nc"]

    shared = _prep_host(inputs)
    feat = np.ascontiguousarray(inputs["feat"].astype(np.float32, copy=False))
    deg = np.ascontiguousarray(
        inputs["deg"].astype(np.float32, copy=False).reshape(B, DEG, 64))

    in_maps = []
    for i in range(N_CORES):
        m = dict(shared)
        m["feat"] = feat[i * BC:(i + 1) * BC]
        m["deg"] = deg[i * BC:(i + 1) * BC]
        in_maps.append(m)

    res = run_bass_kernel_spmd(nc, in_maps, core_ids=list(range(N_CORES)))
    out = np.concatenate([res.results[i]["out"] for i in range(N_CORES)], axis=0)
    return out.astype(np.float32)
